# revision 74
# baseline (speedup 1.0000x reference)
"""CACombiner Trainium2 kernel: conv-projected efficient attention + FFN.

Data-parallel over batch: 8 batch elements -> 8 NeuronCores, identical SPMD
program per core. Attention path (q/k/v projections, ctx, and the fused
Wr@ctx reprojection) runs in fp8e4 with DoubleRow matmuls (2 k-tiles per
instruction, 0.5 cycles/row); the FFN runs in bf16. LayerNorms are fused:
LN1 mean/var via PE row-matmuls + gpsimd partition-broadcast, LN2 likewise
with g2/b2 folded on host.
"""
import sys
sys.path.insert(0, "/opt/trn_rl_repo")
from contextlib import ExitStack

import numpy as np

import concourse.bass as bass
import concourse.tile as tile
from concourse import mybir, bacc
from concourse.bass_utils import run_bass_kernel_spmd
from concourse.alu_op_type import AluOpType

F32 = mybir.dt.float32
F32R = mybir.dt.float32r
BF16 = mybir.dt.bfloat16
F8 = mybir.dt.float8e4
AFT = mybir.ActivationFunctionType
DR = mybir.MatmulPerfMode.DoubleRow

# Every activation this kernel uses (Exp, Ln, Relu, Copy, Square, Identity)
# lives together in one hardware activation-table set. The default chooser
# picks the first set containing each function, which alternates sets and
# inserts a 1.3us LoadActFuncSet per switch. Narrow the chooser's view so the
# all-inclusive set is the unique provider (names/indexes preserved, so the
# emitted act_func_set_id still refers to the true table).
_OUR_FUNCS = {AFT.Exp, AFT.Ln, AFT.Relu, AFT.Copy, AFT.Square, AFT.Identity}


def _patch_act_tables():
    import concourse.hw_specs as hw_specs
    import concourse.bacc as bacc_mod
    orig = hw_specs.get_activation_tables
    if getattr(hw_specs, "_cac_patched", False):
        return

    def patched(arch):
        t = orig(arch)
        keep = None
        for name, s in t.items():
            if _OUR_FUNCS <= s:
                keep = name
                break
        if keep is None:
            return t
        return {name: (s if name == keep else s - _OUR_FUNCS)
                for name, s in t.items()}

    hw_specs.get_activation_tables = patched
    bacc_mod.get_activation_tables = patched
    hw_specs._cac_patched = True

B, C, L = 8, 512, 4096
H, DK = 8, 64
EPS = 1e-5
CC = C // 128            # 4 channel chunks
NT = L // 512            # 8 outer l-tiles
SW = 32.0                # fp8 scale for Wq/Wk/Wv
SQ = 64.0                # fp8 scale for softmaxed q
SC = 256.0               # fp8 scale for W_comb = Wr @ ctx

_CACHE = {}
LAST_RESULT = None


def _build_program(flags):
    bq_nz, br_nz, b1_nz, b2_nz, be2_nz = flags
    _patch_act_tables()
    nc = bacc.Bacc("TRN2", target_bir_lowering=False, debug=False)

    def din(name, shape, dtype):
        return nc.dram_tensor(name, list(shape), dtype, kind="ExternalInput").ap()

    z1_8d = din("z1_8", (128, CC, L), F8)
    z2_8d = din("z2_8", (128, CC, L), F8)
    z1res_d = din("z1res", (128, CC, L), BF16)
    Wq8T_d = din("Wq8T", (128, CC, 512), F8)
    Wkv8T_d = din("Wkv8T", (128, CC, 1024), F8)
    hm8_d = din("hm8", (128, CC, 16), F8)
    hm64_d = din("hm64", (8, CC, 128), F32R)
    WrTb_d = din("WrTb", (128, CC, 512), BF16)
    W1gTb_d = din("W1gTb", (128, CC, 1024), BF16)
    W2gTb_d = din("W2gTb", (128, 8, 512), BF16)
    u2ct_d = din("u2ct", (128, 8), BF16)
    inv512_d = din("inv512", (128, 1), BF16)
    ivg8_d = din("ivg8", (128, CC, 16), F8)
    ones8p_d = din("ones8p", (128, CC, 16), F8)
    g2c_d = din("g2c", (128, CC), F32)
    identb_d = din("identb", (128, 128), BF16)
    eps_d = din("epsA", (1, 1), F32)
    ones_row_d = din("ones_row", (1, 512), F32R)
    bq_r_d = din("bq_r", (1, 512), F32R)
    br_c_d = din("br_c", (128, CC), F32)
    w1bb_r_d = din("w1bb_r", (1, 1024), F32R)
    g2b2_r_d = din("g2b2_r", (1, 512), F32R)
    be2_c_d = din("be2_c", (128, CC), F32)
    outd = nc.dram_tensor("out", [C, L], BF16, kind="ExternalOutput").ap()

    mm = nc.tensor.matmul
    tt = nc.vector.tensor_tensor
    ts = nc.vector.tensor_scalar
    stt = nc.vector.scalar_tensor_tensor
    act = nc.scalar.activation
    gp = nc.gpsimd

    with tile.TileContext(nc) as tc, ExitStack() as ctx:
        cpool = ctx.enter_context(tc.tile_pool(name="consts", bufs=1))

        def const_tile(shape, dtype, src, tag):
            t = cpool.tile(list(shape), dtype, tag=tag, name=tag)
            nc.sync.dma_start(t[:], src)
            return t

        # phase-1 weights first so the first q matmuls aren't queued behind
        # the big FFN weight transfers; the rest loads during phase 1
        Wq8T = const_tile((128, CC, 512), F8, Wq8T_d, "Wq8T")
        hm8 = const_tile((128, CC, 16), F8, hm8_d, "hm8")
        hm64 = const_tile((8, CC, 128), F32R, hm64_d, "hm64")
        identb = const_tile((128, 128), BF16, identb_d, "identb")
        epsA = const_tile((1, 1), F32, eps_d, "epsA")
        ones_row = const_tile((1, 512), F32R, ones_row_d, "ones_row")
        if bq_nz:
            bq_r = const_tile((1, 512), F32R, bq_r_d, "bq_r")

        def load_late_consts():
            c = {}
            c["WrTb"] = const_tile((128, CC, 512), BF16, WrTb_d, "WrTb")
            c["W1gTb"] = const_tile((128, CC, 1024), BF16, W1gTb_d, "W1gTb")
            c["W2gTb"] = const_tile((128, 8, 512), BF16, W2gTb_d, "W2gTb")
            c["u2ct"] = const_tile((128, 8), BF16, u2ct_d, "u2ct")
            c["inv512"] = const_tile((128, 1), BF16, inv512_d, "inv512")
            c["ivg8"] = const_tile((128, CC, 16), F8, ivg8_d, "ivg8")
            c["ones8p"] = const_tile((128, CC, 16), F8, ones8p_d, "ones8p")
            c["g2c"] = const_tile((128, CC), F32, g2c_d, "g2c")
            if br_nz:
                c["br_c"] = const_tile((128, CC), F32, br_c_d, "br_c")
            if b1_nz:
                c["w1bb_r"] = const_tile((1, 1024), F32R, w1bb_r_d, "w1bb_r")
            if b2_nz:
                c["g2b2_r"] = const_tile((1, 512), F32R, g2b2_r_d, "g2b2_r")
            if be2_nz:
                c["be2_c"] = const_tile((128, CC), F32, be2_c_d, "be2_c")
            return c

        # persistent across phases
        qsm8 = cpool.tile([128, CC, L], F8, tag="qsm8", name="qsm8")
        WcT8 = cpool.tile([128, CC, 512], F8, tag="WcT8", name="WcT8")

        # ------------- Phase 1: q softmax (channels-first) + k/v + ctx -------------
        with ExitStack() as p1:
            ps_ctx = p1.enter_context(tc.tile_pool(name="ps_ctx", bufs=1, space="PSUM"))
            ctxa = ps_ctx.tile([128, CC, 129], F32, tag="ctxa", name="ctxa")

            p1i = p1.enter_context(ExitStack())
            lp1 = p1i.enter_context(tc.tile_pool(name="lp1", bufs=3))
            lpk = p1i.enter_context(tc.tile_pool(name="lpk", bufs=1))
            ps_q = p1i.enter_context(tc.tile_pool(name="ps_q", bufs=2, space="PSUM"))
            ps_m = p1i.enter_context(tc.tile_pool(name="ps_m", bufs=2, space="PSUM"))
            ps_k = p1i.enter_context(tc.tile_pool(name="ps_k", bufs=2, space="PSUM"))

            # persistent Ek/vT pair tiles (2 rotating pairs); the ones-columns
            # of vT are set once and never overwritten
            Ek2s = [lpk.tile([128, 2, 512], F8, tag=f"Ek2{i}", name=f"Ek2{i}")
                    for i in range(2)]
            # chunk blocks padded 129 -> 144 so the DoubleRow rhs outer stride
            # (2*288... the slot stride 576 and block step 144) is 16-aligned
            vT2s = [lpk.tile([128, 2, 576], F8, tag=f"vT2{i}", name=f"vT2{i}")
                    for i in range(2)]
            for i in range(2):
                nc.vector.memset(
                    vT2s[i][:].rearrange("p t (pr x) -> p t pr x", x=144)[:, :, :, 128:129],
                    1.0)

            # q-section for tile `ot` is emitted in 3 pieces interleaved into
            # the kv/ctx loop of tile ot-1 so the Eq-activation latency never
            # stalls PE: piece 0 = DMA + q matmuls for oc 0,1; piece 1 = exps
            # for oc 0,1 + q matmuls oc 2,3; piece 2 = exps oc 2,3 + per-head
            # sums (DoubleRow mask matmul into a spare qps-ring slice).
            qstate = {}

            def q_piece(ot, k):
                sl = slice(ot * 512, (ot + 1) * 512)
                if k == 0:
                    st = qstate[ot] = {}
                    st["z1c"] = lp1.tile([128, CC, 512], F8, tag="z1c", name="z1c")
                    nc.sync.dma_start(st["z1c"][:], z1_8d[:, :, sl])
                    st["z2c"] = lp1.tile([128, CC, 512], F8, tag="z2c", name="z2c")
                    nc.sync.dma_start(st["z2c"][:], z2_8d[:, :, sl])
                    st["qsmE"] = lp1.tile([128, CC, 512], F8, tag="qsmE", name="qsmE")
                    st["qp"] = []
                st = qstate[ot]
                if k in (0, 1):
                    for i in range(2):
                        oc = 2 * k + i
                        os_ = slice(oc * 128, (oc + 1) * 128)
                        qp = ps_q.tile([128, 512], F32, tag="qps", name="qp")
                        st["qp"].append(qp)
                        mm(qp[:], Wq8T[:, 0:2, os_], st["z1c"][:, 0:2, :],
                           start=True, stop=False, perf_mode=DR)
                        mm(qp[:], Wq8T[:, 2:4, os_], st["z1c"][:, 2:4, :],
                           start=False, stop=not bq_nz, perf_mode=DR)
                        if bq_nz:
                            mm(qp[:], bq_r[:, os_], ones_row[:],
                               start=False, stop=True)
                if k == 1:
                    for oc in (0, 1):
                        act(st["qsmE"][:, oc, :], st["qp"][oc][:], AFT.Exp,
                            scale=1.0 / SW)
                if k == 2:
                    for oc in (2, 3):
                        act(st["qsmE"][:, oc, :], st["qp"][oc][:], AFT.Exp,
                            scale=1.0 / SW)
                    sqt = ps_q.tile([128, 512], F32, tag="qps", name="sqt")
                    st["sqt"] = sqt
                    mm(sqt[0:16, :], hm8[:, 0:2, :], st["qsmE"][:, 0:2, :],
                       start=True, stop=False, perf_mode=DR, skip_group_check=True)
                    mm(sqt[0:16, :], hm8[:, 2:4, :], st["qsmE"][:, 2:4, :],
                       start=False, stop=True, perf_mode=DR, skip_group_check=True)
                    rqf = lp1.tile([8, 512], F32R, tag="rqf", name="rqf")
                    st["rqf"] = rqf
                    with nc.allow_low_precision(reason="f32r row for broadcast mm"):
                        nc.vector.reciprocal(rqf[:], sqt[0:8, :])

            q_piece(0, 0)
            Wkv8T = const_tile((128, CC, 1024), F8, Wkv8T_d, "Wkv8T")
            q_piece(0, 1)
            late = load_late_consts()
            WrTb, W1gTb, W2gTb = late["WrTb"], late["W1gTb"], late["W2gTb"]
            u2ct, inv512, ivg8, g2c = (late["u2ct"], late["inv512"],
                                       late["ivg8"], late["g2c"])
            ones8p = late["ones8p"]
            br_c = late.get("br_c")
            w1bb_r = late.get("w1bb_r")
            g2b2_r = late.get("g2b2_r")
            be2_c = late.get("be2_c")
            q_piece(0, 2)

            for ot in range(NT):
                sl = slice(ot * 512, (ot + 1) * 512)
                st = qstate[ot]
                z2c, qsmE, rqf = st["z2c"], st["qsmE"], st["rqf"]
                for s in range(4):
                    ls = slice(s * 128, (s + 1) * 128)
                    slot = s % 2
                    pair = (ot * 2 + s // 2) % 2
                    Ek2, vT2 = Ek2s[pair], vT2s[pair]
                    pr = s
                    # qsm8 = qsmE * (64/Sq) broadcast per head
                    rqbt = ps_m.tile([128, 512], F32, tag="mps", name="rqbt")
                    mm(rqbt[:], hm64[:, pr, :], rqf[:],
                       start=True, stop=True)
                    tt(qsm8[:, pr, sl], qsmE[:, pr, :], rqbt[:],
                       AluOpType.mult)
                    kps = ps_k.tile([128, 512], F32, tag="kps", name="kps")
                    for p in (0, 2):
                        mm(kps[:], z2c[:, p:p + 2, ls], Wkv8T[:, p:p + 2, 0:512],
                           start=(p == 0), stop=(p == 2), perf_mode=DR)
                    vps = ps_m.tile([128, 512], F32, tag="mps", name="vps")
                    for p in (0, 2):
                        mm(vps[:], z2c[:, p:p + 2, ls], Wkv8T[:, p:p + 2, 512:1024],
                           start=(p == 0), stop=(p == 2), perf_mode=DR)
                    act(Ek2[:, slot, :], kps[:], AFT.Exp, scale=1.0 / SW)
                    vdst = vT2[:, slot, :].rearrange("p (pr x) -> p pr x", x=144)[:, :, 0:128]
                    vsrc = vps[:].rearrange("p (pr x) -> p pr x", x=128)
                    if s == 3:
                        act(vdst, vsrc, AFT.Copy)
                    else:
                        nc.vector.tensor_copy(vdst, vsrc)
                    if slot == 1:
                        first = (ot == 0 and s == 1)
                        last = (ot == NT - 1 and s == 3)
                        for pr2 in range(CC):
                            mm(ctxa[:, pr2, :], Ek2[:, :, pr2 * 128:(pr2 + 1) * 128],
                               vT2[:, :, pr2 * 144:pr2 * 144 + 129],
                               start=first, stop=last, perf_mode=DR,
                               skip_group_check=True)
                    if ot + 1 < NT and s < 3:
                        q_piece(ot + 1, s)
                if ot in qstate:
                    del qstate[ot]

            # finalize: normalize ctx rows, build W_combT = ctx_bd^T @ Wr^T in fp8
            p1i.close()
            with ExitStack() as fz:
                ft = fz.enter_context(tc.tile_pool(name="ft", bufs=1))
                ps_t = fz.enter_context(tc.tile_pool(name="ps_t", bufs=2, space="PSUM"))
                rs_l, cbd_l, tps_l, cT_l = [], [], [], []
                for pr in range(CC):
                    rs = ft.tile([128, 1], F32, tag=f"rs{pr}", name=f"rs{pr}")
                    nc.vector.reciprocal(rs[:], ctxa[:, pr, 128:129])
                    rs_l.append(rs)
                    cbd = ft.tile([128, 128], BF16, tag=f"cbd{pr}", name=f"cbd{pr}")
                    nc.vector.memset(cbd[:], 0.0)
                    cbd_l.append(cbd)
                for pr in range(CC):
                    ts(cbd_l[pr][0:64, 0:64], ctxa[0:64, pr, 0:64], rs_l[pr][0:64, :],
                       1.0 / SW, AluOpType.mult, AluOpType.mult)
                    ts(cbd_l[pr][64:128, 64:128], ctxa[64:128, pr, 64:128],
                       rs_l[pr][64:128, :], 1.0 / SW, AluOpType.mult, AluOpType.mult)
                for pr in range(CC):
                    tps = ps_t.tile([128, 128], BF16, tag="tps")
                    nc.tensor.transpose(tps[:], cbd_l[pr][:], identb[:])
                    tps_l.append(tps)
                    cT = ft.tile([128, 128], BF16, tag=f"cT{pr}", name=f"cT{pr}")
                    nc.vector.tensor_copy(cT[:], tps[:])
                    cT_l.append(cT)
                for pr in range(CC):
                    wcps = ps_t.tile([128, 512], F32, tag="wcps")
                    mm(wcps[:], cT_l[pr][:], WrTb[:, pr, :], start=True, stop=True)
                    act(WcT8[:, pr, :], wcps[:], AFT.Copy, scale=SC)

        # ------------- Phase 2: z = Wc qsm + z1, LN1, FFN, LN2 -------------
        # Software-pipelined: stage A (z + LN1 stats + xn) runs one tile ahead
        # of stage B (FFN + LN2 + output) so B's long FFN matmul stretch hides
        # A's LN1 latency chain and A(t+1)'s z matmuls hide B(t)'s LN2 tail.
        with ExitStack() as p2:
            lp2 = p2.enter_context(tc.tile_pool(name="lp2", bufs=2))
            lpx = p2.enter_context(tc.tile_pool(name="lpx", bufs=3))
            lpr = p2.enter_context(tc.tile_pool(name="lpr", bufs=2))
            lpe = p2.enter_context(tc.tile_pool(name="lpe", bufs=3))
            ps_z = p2.enter_context(tc.tile_pool(name="ps_z", bufs=2, space="PSUM"))
            ps_f = p2.enter_context(tc.tile_pool(name="ps_f", bufs=2, space="PSUM"))
            ps_f2 = p2.enter_context(tc.tile_pool(name="ps_f2", bufs=2, space="PSUM"))
            ps_row = p2.enter_context(tc.tile_pool(name="ps_row", bufs=2, space="PSUM"))

            def stage_a(lt):
                sl = slice(lt * 512, (lt + 1) * 512)
                z1r = lp2.tile([128, CC, 512], BF16, tag="z1r", name="z1r")
                nc.sync.dma_start(z1r[:], z1res_d[:, :, sl])
                rows = ps_row.tile([128, 512], F32, tag="rows", name="rows")

                zb = lp2.tile([128, CC, 512], BF16, tag="zb", name="zb")
                zsq = lp2.tile([128, CC, 512], F8, tag="zsq", name="zsq")
                for oc in range(CC):
                    os_ = slice(oc * 128, (oc + 1) * 128)
                    zps = ps_z.tile([128, 512], F32, tag="zps", name="zps")
                    mm(zps[:], WcT8[:, 0:2, os_], qsm8[:, 0:2, sl],
                       start=True, stop=False, perf_mode=DR)
                    mm(zps[:], WcT8[:, 2:4, os_], qsm8[:, 2:4, sl],
                       start=False, stop=True, perf_mode=DR)
                    stt(zb[:, oc, :], zps[:], 1.0 / (SC * SQ), z1r[:, oc, :],
                        AluOpType.mult, AluOpType.add)
                    if br_nz:
                        gp.tensor_scalar(zb[:, oc, :], zb[:, oc, :],
                                         br_c[:, oc:oc + 1], None, AluOpType.add)
                    act(zsq[:, oc, :], zb[:, oc, :], AFT.Square)
                # mean row at partition 32 (bf16 matmul may target 32); the
                # fp8 DoubleRow square-sum must target partition 0
                for oc in range(CC):
                    mm(rows[32:33, :], inv512[:], zb[:, oc, :],
                       start=(oc == 0), stop=(oc == CC - 1), skip_group_check=True)
                mm(rows[0:1, :], ones8p[:, 0:2, 0:1], zsq[:, 0:2, :],
                   start=True, stop=False, perf_mode=DR, skip_group_check=True)
                mm(rows[0:1, :], ones8p[:, 2:4, 0:1], zsq[:, 2:4, :],
                   start=False, stop=True, perf_mode=DR, skip_group_check=True)

                # LN1 rows: 1/sigma = exp(-0.5 ln(var+eps)) keeps every ACT op
                # in the same activation-table set (no table reloads)
                musq = lpr.tile([1, 512], BF16, tag="musq", name="musq")
                act(musq[:], rows[32:33, :], AFT.Square)
                varb = lpr.tile([1, 512], BF16, tag="varb", name="varb")
                stt(varb[:], rows[0:1, :], 1.0 / 512.0, musq[:],
                    AluOpType.mult, AluOpType.subtract)
                lnv = lpr.tile([1, 512], F32, tag="lnv", name="lnv")
                act(lnv[:], varb[:], AFT.Ln, bias=epsA[0:1, :])
                invbr = lpr.tile([1, 512], BF16, tag="invbr", name="invbr")
                act(invbr[:], lnv[:], AFT.Exp, scale=-0.5)
                numur = lpr.tile([1, 512], BF16, tag="numur", name="numur")
                stt(numur[:], rows[32:33, :], -1.0, invbr[:], AluOpType.mult,
                    AluOpType.mult)
                invsb = lp2.tile([128, 512], BF16, tag="invsb", name="invsb")
                gp.partition_broadcast(invsb[:], invbr[:])
                numub = lp2.tile([128, 512], BF16, tag="numub", name="numub")
                gp.partition_broadcast(numub[:], numur[:])

                xn = lpx.tile([128, CC, 512], BF16, tag="xn", name="xn")
                for oc in range(CC):
                    tt(xn[:, oc, :], zb[:, oc, :], invsb[:], AluOpType.mult)
                    tt(xn[:, oc, :], xn[:, oc, :], numub[:], AluOpType.add)
                return sl, rows, xn

            def b2_chunk(st, oc, s1, sq2, drain=False):
                sl2, heh2, negm2b2 = st
                os_ = slice(oc * 128, (oc + 1) * 128)
                f2ps = ps_f2.tile([128, 512], F32, tag="f2ps", name="f2ps")
                for j in range(8):
                    mm(f2ps[:], W2gTb[:, j, os_], heh2[j // 4][:, j % 4, :],
                       start=(j == 0), stop=(j == 7 and not b2_nz))
                if b2_nz:
                    mm(f2ps[:], g2b2_r[:, os_], ones_row[:], start=False, stop=True)
                stt(s1[:, oc, :], negm2b2[:], g2c[:, oc:oc + 1], f2ps[:],
                    AluOpType.mult, AluOpType.add)
                # during the pipeline drain DVE is idle and Pool's slow TT
                # would sit on the critical path
                sqeng = nc.vector if drain else gp
                sqeng.tensor_tensor(sq2[:, oc, :], s1[:, oc, :], s1[:, oc, :],
                                    AluOpType.mult)

            def b2_tail(st, s1, sq2):
                sl2, heh2, negm2b2 = st
                # variance row via fp8 DoubleRow (ivg8 = 1/g2^2 in col 0);
                # the 1/512 is folded into the Ln scale
                e2t = ps_f.tile([128, 512], F32, tag="fps", name="e2t")
                mm(e2t[0:1, :], ivg8[:, 0:2, 0:1], sq2[:, 0:2, :],
                   start=True, stop=False, perf_mode=DR, skip_group_check=True)
                mm(e2t[0:1, :], ivg8[:, 2:4, 0:1], sq2[:, 2:4, :],
                   start=False, stop=True, perf_mode=DR, skip_group_check=True)

                ln2v = lpr.tile([1, 512], F32, tag="ln2v", name="ln2v")
                act(ln2v[:], e2t[0:1, :], AFT.Ln, scale=1.0 / 512.0,
                    bias=epsA[0:1, :])
                inv2br = lpr.tile([1, 512], BF16, tag="inv2br", name="inv2br")
                act(inv2br[:], ln2v[:], AFT.Exp, scale=-0.5)
                invs2b = lp2.tile([128, 512], BF16, tag="invs2b", name="invs2b")
                gp.partition_broadcast(invs2b[:], inv2br[:])

                for oc in range(CC):
                    yo = lp2.tile([128, 512], BF16, tag=f"yo{oc}", name=f"yo{oc}")
                    tt(yo[:], s1[:, oc, :], invs2b[:], AluOpType.mult)
                    if be2_nz:
                        ts(yo[:], yo[:], be2_c[:, oc:oc + 1], None, AluOpType.add)
                    nc.sync.dma_start(outd[oc * 128:(oc + 1) * 128, sl2], yo[:])

            def stage_b(a_st, b_st):
                """FFN1+ELU for tile a_st, with the previous tile's FFN2
                oc-chunks interleaved between FFN1 j-pairs so PE always has
                independent matmuls while the ELU chain drains."""
                if b_st is not None:
                    s1 = lp2.tile([128, CC, 512], BF16, tag="s1", name="s1")
                    sq2 = lp2.tile([128, CC, 512], F8, tag="sq2", name="sq2")
                if a_st is None:
                    for oc in range(CC):
                        b2_chunk(b_st, oc, s1, sq2, drain=True)
                    b2_tail(b_st, s1, sq2)
                    return None
                sl, rows, xn = a_st
                heh = [lp2.tile([128, 4, 512], BF16, tag=f"he{h}", name=f"he{h}")
                       for h in range(2)]
                for j in range(8):
                    fps = ps_f.tile([128, 512], F32, tag="fps", name="fps")
                    js = slice(j * 128, (j + 1) * 128)
                    for cc in range(CC):
                        mm(fps[:], W1gTb[:, cc, js], xn[:, cc, :],
                           start=(cc == 0), stop=(cc == CC - 1 and not b1_nz))
                    if b1_nz:
                        mm(fps[:], w1bb_r[:, js], ones_row[:], start=False, stop=True)
                    Eb = lpe.tile([128, 512], BF16, tag="Eb", name="Eb")
                    act(Eb[:], fps[:], AFT.Exp)
                    ts(Eb[:], Eb[:], 1.0, -1.0, AluOpType.min, AluOpType.add)
                    if j % 2 == 0:
                        # elu in one DVE pass: max(h,0) + (min(exp(h),1)-1)
                        stt(heh[j // 4][:, j % 4, :], fps[:], 0.0, Eb[:],
                            AluOpType.max, AluOpType.add)
                    else:
                        hp = lpe.tile([128, 512], BF16, tag="hp", name="hp")
                        act(hp[:], fps[:], AFT.Relu)
                        tt(heh[j // 4][:, j % 4, :], hp[:], Eb[:], AluOpType.add)
                    if b_st is not None and j % 2 == 1:
                        b2_chunk(b_st, j // 2, s1, sq2)
                for j in range(8):
                    mm(rows[64:65, :], u2ct[:, j:j + 1], heh[j // 4][:, j % 4, :],
                       start=(j == 0), stop=(j == 7), skip_group_check=True)
                negm2 = lpr.tile([1, 512], BF16, tag="negm2", name="negm2")
                ts(negm2[:], rows[64:65, :], -1.0, -B2MEAN_PLACEHOLDER,
                   AluOpType.mult, AluOpType.add)
                negm2b = lp2.tile([128, 512], BF16, tag="negm2b", name="negm2b")
                gp.partition_broadcast(negm2b[:], negm2[:])
                if b_st is not None:
                    b2_tail(b_st, s1, sq2)
                return sl, heh, negm2b

            pa, pb = None, None
            for lt in range(NT):
                cur = stage_a(lt)
                if pa is not None:
                    pb = stage_b(pa, pb)
                pa = cur
            pb = stage_b(pa, pb)
            stage_b(None, pb)

    nc.compile()
    return nc


def _prep_consts(Wq, bq, Wk, bk, Wv, bv, Wr, br, g1, be1, W1, b1, W2, b2, g2, be2):
    import ml_dtypes
    f = np.float32
    F8NP = ml_dtypes.float8_e4m3
    BFNP = ml_dtypes.bfloat16

    def chunkT(a, n):          # [n*128, m] -> [128, n, m]
        return np.ascontiguousarray(a.reshape(n, 128, -1).transpose(1, 0, 2))

    def colsT(v, n):           # [n*128] -> [128, n]
        return np.ascontiguousarray(v.reshape(n, 128).T)

    WqT = np.ascontiguousarray(Wq.T, dtype=f)
    WkvT = np.concatenate([Wk.T, Wv.T], axis=1).astype(f)
    WrT = np.ascontiguousarray(Wr.T, dtype=f)
    W1g = (W1 * g1[None, :]).astype(f)
    W1gT = np.ascontiguousarray(W1g.T)
    W2g = (W2 * g2[:, None]).astype(f)
    W2gT = np.ascontiguousarray(W2g.T)
    w1bb = (W1 @ be1 + b1).astype(f)
    u2 = (W2.sum(axis=0) / 512.0).astype(f)
    ivg = (1.0 / (g2 * g2)).astype(f)          # 1/512 folded into Ln scale
    b2mean = float(np.mean(b2))
    br_eff = (br + Wr @ bv).astype(f)
    ivg8 = np.zeros((128, CC, 16), dtype=f)
    ivg8[:, :, 0] = colsT(ivg, CC)
    ones8p = np.zeros((128, CC, 16), dtype=f)
    ones8p[:, :, 0] = 1.0

    # head mask: channel (cc, p) -> global k-channel cc*128+p -> head //64
    chan = (np.arange(CC)[None, :] * 128 + np.arange(128)[:, None])  # [128, CC]
    head = chan // DK                                                # [128, CC]
    hm8 = np.zeros((128, CC, 16), dtype=f)   # padded to 16 cols for DoubleRow
    for hh in range(8):
        hm8[:, :, hh] = (head == hh)
    hm64 = np.zeros((8, CC, 128), dtype=f)
    for pr in range(CC):
        for hh in range(8):
            hm64[hh, pr, :] = 64.0 * (head[:, pr] == hh)

    consts = {
        "Wq8T": chunkT(WqT * SW, CC).astype(F8NP),
        "Wkv8T": chunkT(WkvT * SW, CC).astype(F8NP),
        "hm8": hm8.astype(F8NP),
        "hm64": hm64,
        "WrTb": chunkT(WrT, CC).astype(BFNP),
        "W1gTb": chunkT(W1gT, CC).astype(BFNP),
        "W2gTb": chunkT(W2gT, 8).astype(BFNP),
        "u2ct": colsT(u2, 8).astype(BFNP),
        "inv512": np.full((128, 1), 1.0 / 512.0, dtype=f).astype(BFNP),
        "ivg8": ivg8.astype(F8NP),
        "ones8p": ones8p.astype(F8NP),
        "g2c": colsT(g2.astype(f), CC),
        "identb": np.eye(128, dtype=f).astype(BFNP),
        "epsA": np.full((1, 1), EPS, dtype=f),
        "ones_row": np.ones((1, 512), dtype=f),
        "bq_r": bq.reshape(1, 512).astype(f),
        "br_c": colsT(br_eff, CC),
        "w1bb_r": w1bb.reshape(1, 1024).astype(f),
        "g2b2_r": (g2 * b2).reshape(1, 512).astype(f),
        "be2_c": colsT(be2.astype(f), CC),
    }
    flags = (bool(np.any(bq)), bool(np.any(br_eff)), bool(np.any(w1bb)),
             bool(np.any(b2)), bool(np.any(be2)))
    return consts, b2mean, flags


def kernel(**inputs):
    global LAST_RESULT, B2MEAN_PLACEHOLDER
    import ml_dtypes
    F8NP = ml_dtypes.float8_e4m3
    BFNP = ml_dtypes.bfloat16
    z1 = np.asarray(inputs["z1"], dtype=np.float32)
    z2 = np.asarray(inputs["z2"], dtype=np.float32)
    consts, b2mean, flags = _prep_consts(
        *[np.asarray(inputs[k], dtype=np.float32) for k in
          ["Wq", "bq", "Wk", "bk", "Wv", "bv", "Wr", "br", "g1", "be1",
           "W1", "b1", "W2", "b2", "g2", "be2"]])

    key = ("prog", flags, round(b2mean * 1e9))
    if key not in _CACHE:
        B2MEAN_PLACEHOLDER = b2mean
        _CACHE.clear()
        _CACHE[key] = _build_program(flags)
    nc = _CACHE[key]

    def rearr(a):             # [C, L] -> [128, CC, L]
        return np.ascontiguousarray(a.reshape(CC, 128, L).transpose(1, 0, 2))

    in_maps = []
    for b in range(B):
        m = dict(consts)
        m["z1_8"] = rearr(z1[b]).astype(F8NP)
        m["z2_8"] = rearr(z2[b]).astype(F8NP)
        m["z1res"] = rearr(z1[b]).astype(BFNP)
        in_maps.append(m)

    import os
    trace = bool(int(os.environ.get("KERNEL_TRACE", "0")))
    res = run_bass_kernel_spmd(nc, in_maps, list(range(B)), trace=trace)
    LAST_RESULT = res
    out = np.stack([np.asarray(res.results[b]["out"]).astype(np.float32)
                    for b in range(B)], axis=0)
    return out


B2MEAN_PLACEHOLDER = 0.0


# revision 81
# speedup vs baseline: 1.0060x; 1.0060x over previous
"""CACombiner Trainium2 kernel: conv-projected efficient attention + FFN.

Data-parallel over batch: 8 batch elements -> 8 NeuronCores, identical SPMD
program per core. ~247us per core (vs 756us baseline), rel err ~8e-3.

Structure:
- Attention path entirely in fp8e4 DoubleRow matmuls (2 k-tiles/instr, 0.5
  cycles/row): q/k/v projections (host-quantized z1/z2/weights), per-head
  softmax sums via a padded mask matmul, ctx accumulation over l-pairs, and
  the z reprojection against W_comb = Wr @ (ctx/S) built once on device.
  FFN stays bf16 (fp8 FFN exceeds the 2e-2 tolerance; measured 3.7e-2).
- Algebraic folds: bk cancels in the l-softmax; bv flows through attention
  into br_eff = br + Wr@bv; g1/be1/b1 into W1g/w1bb; g2/b2 into W2g and the
  LN2 rows; biases that are zero skip their device ops (flags).
- LayerNorms: stats as PE row-matmuls into one PSUM tile at partitions 0/32
  (variance sums via fp8-DR with ones/1/g2^2 lhsT, 1/512 folded into Ln
  scale); 1/sigma = exp(-0.5*ln(var+eps)) so every activation (Exp/Ln/Relu/
  Copy/Square) lives in ONE hw act-table set (see _patch_act_tables) -> no
  LoadActFuncSet switches; gpsimd partition_broadcast for row->tile.
- Phase 1 (per 512-l tile): q channels-first + interleaved q-pieces of the
  NEXT tile inside the kv/ctx subloop; phase 2: 3-stage software pipeline
  A(t+1)=z+LN1+xn | B1(t)=FFN1+ELU+mu2 | B2(t-1)=FFN2+LN2+out, with B2
  oc-chunks interleaved between B1 j-pairs so PE never drains.
- HW codegen constraints honored: GPSIMD cannot access PSUM; DoubleRow needs
  16B-aligned outer strides (hm8 padded to 16 cols, vT blocks 129->144) and
  dst partition 0; f32r matmul inputs need f32r-rounded producers; DVE ops
  may read at most one PSUM operand.
"""
import sys
sys.path.insert(0, "/opt/trn_rl_repo")
from contextlib import ExitStack

import numpy as np

import concourse.bass as bass
import concourse.tile as tile
from concourse import mybir, bacc
from concourse.bass_utils import run_bass_kernel_spmd
from concourse.alu_op_type import AluOpType

F32 = mybir.dt.float32
F32R = mybir.dt.float32r
BF16 = mybir.dt.bfloat16
F8 = mybir.dt.float8e4
AFT = mybir.ActivationFunctionType
DR = mybir.MatmulPerfMode.DoubleRow

# Every activation this kernel uses (Exp, Ln, Relu, Copy, Square, Identity)
# lives together in one hardware activation-table set. The default chooser
# picks the first set containing each function, which alternates sets and
# inserts a 1.3us LoadActFuncSet per switch. Narrow the chooser's view so the
# all-inclusive set is the unique provider (names/indexes preserved, so the
# emitted act_func_set_id still refers to the true table).
_OUR_FUNCS = {AFT.Exp, AFT.Ln, AFT.Relu, AFT.Copy, AFT.Square, AFT.Identity}


def _patch_act_tables():
    import concourse.hw_specs as hw_specs
    import concourse.bacc as bacc_mod
    orig = hw_specs.get_activation_tables
    if getattr(hw_specs, "_cac_patched", False):
        return

    def patched(arch):
        t = orig(arch)
        keep = None
        for name, s in t.items():
            if _OUR_FUNCS <= s:
                keep = name
                break
        if keep is None:
            return t
        return {name: (s if name == keep else s - _OUR_FUNCS)
                for name, s in t.items()}

    hw_specs.get_activation_tables = patched
    bacc_mod.get_activation_tables = patched
    hw_specs._cac_patched = True

B, C, L = 8, 512, 4096
H, DK = 8, 64
EPS = 1e-5
CC = C // 128            # 4 channel chunks
NT = L // 512            # 8 outer l-tiles
SW = 32.0                # fp8 scale for Wq/Wk/Wv
SQ = 64.0                # fp8 scale for softmaxed q
SC = 256.0               # fp8 scale for W_comb = Wr @ ctx

_CACHE = {}
LAST_RESULT = None


def _build_program(flags):
    bq_nz, br_nz, b1_nz, b2_nz, be2_nz = flags
    _patch_act_tables()
    nc = bacc.Bacc("TRN2", target_bir_lowering=False, debug=False)

    def din(name, shape, dtype):
        return nc.dram_tensor(name, list(shape), dtype, kind="ExternalInput").ap()

    z1_8d = din("z1_8", (128, CC, L), F8)
    z2_8d = din("z2_8", (128, CC, L), F8)
    z1res_d = din("z1res", (128, CC, L), BF16)
    Wq8T_d = din("Wq8T", (128, CC, 512), F8)
    Wkv8T_d = din("Wkv8T", (128, CC, 1024), F8)
    hm8_d = din("hm8", (128, CC, 16), F8)
    hm64_d = din("hm64", (8, CC, 128), F32R)
    WrTb_d = din("WrTb", (128, CC, 512), BF16)
    W1gTb_d = din("W1gTb", (128, CC, 1024), BF16)
    W2gTb_d = din("W2gTb", (128, 8, 512), BF16)
    u2ct_d = din("u2ct", (128, 8), BF16)
    inv512_d = din("inv512", (128, 1), BF16)
    ivg8_d = din("ivg8", (128, CC, 16), F8)
    ones8p_d = din("ones8p", (128, CC, 16), F8)
    g2c_d = din("g2c", (128, CC), F32)
    identb_d = din("identb", (128, 128), BF16)
    eps_d = din("epsA", (1, 1), F32)
    ones_row_d = din("ones_row", (1, 512), F32R)
    bq_r_d = din("bq_r", (1, 512), F32R)
    br_c_d = din("br_c", (128, CC), F32)
    w1bb_r_d = din("w1bb_r", (1, 1024), F32R)
    g2b2_r_d = din("g2b2_r", (1, 512), F32R)
    be2_c_d = din("be2_c", (128, CC), F32)
    outd = nc.dram_tensor("out", [C, L], BF16, kind="ExternalOutput").ap()

    mm = nc.tensor.matmul
    tt = nc.vector.tensor_tensor
    ts = nc.vector.tensor_scalar
    stt = nc.vector.scalar_tensor_tensor
    act = nc.scalar.activation
    gp = nc.gpsimd

    with tile.TileContext(nc) as tc, ExitStack() as ctx:
        cpool = ctx.enter_context(tc.tile_pool(name="consts", bufs=1))

        def const_tile(shape, dtype, src, tag):
            t = cpool.tile(list(shape), dtype, tag=tag, name=tag)
            nc.sync.dma_start(t[:], src)
            return t

        # phase-1 weights first so the first q matmuls aren't queued behind
        # the big FFN weight transfers; the rest loads during phase 1
        Wq8T = const_tile((128, CC, 512), F8, Wq8T_d, "Wq8T")
        hm8 = const_tile((128, CC, 16), F8, hm8_d, "hm8")
        hm64 = const_tile((8, CC, 128), F32R, hm64_d, "hm64")
        identb = const_tile((128, 128), BF16, identb_d, "identb")
        epsA = const_tile((1, 1), F32, eps_d, "epsA")
        ones_row = const_tile((1, 512), F32R, ones_row_d, "ones_row")
        if bq_nz:
            bq_r = const_tile((1, 512), F32R, bq_r_d, "bq_r")

        def load_late_consts():
            c = {}
            c["WrTb"] = const_tile((128, CC, 512), BF16, WrTb_d, "WrTb")
            c["W1gTb"] = const_tile((128, CC, 1024), BF16, W1gTb_d, "W1gTb")
            c["W2gTb"] = const_tile((128, 8, 512), BF16, W2gTb_d, "W2gTb")
            c["u2ct"] = const_tile((128, 8), BF16, u2ct_d, "u2ct")
            c["inv512"] = const_tile((128, 1), BF16, inv512_d, "inv512")
            c["ivg8"] = const_tile((128, CC, 16), F8, ivg8_d, "ivg8")
            c["ones8p"] = const_tile((128, CC, 16), F8, ones8p_d, "ones8p")
            c["g2c"] = const_tile((128, CC), F32, g2c_d, "g2c")
            if br_nz:
                c["br_c"] = const_tile((128, CC), F32, br_c_d, "br_c")
            if b1_nz:
                c["w1bb_r"] = const_tile((1, 1024), F32R, w1bb_r_d, "w1bb_r")
            if b2_nz:
                c["g2b2_r"] = const_tile((1, 512), F32R, g2b2_r_d, "g2b2_r")
            if be2_nz:
                c["be2_c"] = const_tile((128, CC), F32, be2_c_d, "be2_c")
            return c

        # persistent across phases
        qsm8 = cpool.tile([128, CC, L], F8, tag="qsm8", name="qsm8")
        WcT8 = cpool.tile([128, CC, 512], F8, tag="WcT8", name="WcT8")

        # ------------- Phase 1: q softmax (channels-first) + k/v + ctx -------------
        with ExitStack() as p1:
            ps_ctx = p1.enter_context(tc.tile_pool(name="ps_ctx", bufs=1, space="PSUM"))
            ctxa = ps_ctx.tile([128, CC, 129], F32, tag="ctxa", name="ctxa")

            p1i = p1.enter_context(ExitStack())
            lp1 = p1i.enter_context(tc.tile_pool(name="lp1", bufs=3))
            lpk = p1i.enter_context(tc.tile_pool(name="lpk", bufs=1))
            ps_q = p1i.enter_context(tc.tile_pool(name="ps_q", bufs=2, space="PSUM"))
            ps_m = p1i.enter_context(tc.tile_pool(name="ps_m", bufs=2, space="PSUM"))
            ps_k = p1i.enter_context(tc.tile_pool(name="ps_k", bufs=2, space="PSUM"))

            # persistent Ek/vT pair tiles (2 rotating pairs); the ones-columns
            # of vT are set once and never overwritten
            Ek2s = [lpk.tile([128, 2, 512], F8, tag=f"Ek2{i}", name=f"Ek2{i}")
                    for i in range(3)]
            # chunk blocks padded 129 -> 144 so the DoubleRow rhs outer stride
            # (2*288... the slot stride 576 and block step 144) is 16-aligned
            vT2s = [lpk.tile([128, 2, 576], F8, tag=f"vT2{i}", name=f"vT2{i}")
                    for i in range(3)]
            for i in range(3):
                nc.vector.memset(
                    vT2s[i][:].rearrange("p t (pr x) -> p t pr x", x=144)[:, :, :, 128:129],
                    1.0)

            # q-section for tile `ot` is emitted in 3 pieces interleaved into
            # the kv/ctx loop of tile ot-1 so the Eq-activation latency never
            # stalls PE: piece 0 = DMA + q matmuls for oc 0,1; piece 1 = exps
            # for oc 0,1 + q matmuls oc 2,3; piece 2 = exps oc 2,3 + per-head
            # sums (DoubleRow mask matmul into a spare qps-ring slice).
            qstate = {}

            def q_piece(ot, k):
                sl = slice(ot * 512, (ot + 1) * 512)
                if k == 0:
                    st = qstate[ot] = {}
                    # first tile's inputs ride the Activation DMA queue so
                    # they aren't serialized behind the const transfers
                    dma = nc.scalar.dma_start if ot == 0 else nc.sync.dma_start
                    st["z1c"] = lp1.tile([128, CC, 512], F8, tag="z1c", name="z1c")
                    dma(st["z1c"][:], z1_8d[:, :, sl])
                    st["z2c"] = lp1.tile([128, CC, 512], F8, tag="z2c", name="z2c")
                    dma(st["z2c"][:], z2_8d[:, :, sl])
                    st["qsmE"] = lp1.tile([128, CC, 512], F8, tag="qsmE", name="qsmE")
                    st["qp"] = []
                st = qstate[ot]
                if k in (0, 1):
                    for i in range(2):
                        oc = 2 * k + i
                        os_ = slice(oc * 128, (oc + 1) * 128)
                        qp = ps_q.tile([128, 512], F32, tag="qps", name="qp")
                        st["qp"].append(qp)
                        mm(qp[:], Wq8T[:, 0:2, os_], st["z1c"][:, 0:2, :],
                           start=True, stop=False, perf_mode=DR)
                        mm(qp[:], Wq8T[:, 2:4, os_], st["z1c"][:, 2:4, :],
                           start=False, stop=not bq_nz, perf_mode=DR)
                        if bq_nz:
                            mm(qp[:], bq_r[:, os_], ones_row[:],
                               start=False, stop=True)
                if k == 1:
                    for oc in (0, 1):
                        act(st["qsmE"][:, oc, :], st["qp"][oc][:], AFT.Exp,
                            scale=1.0 / SW)
                if k == 2:
                    for oc in (2, 3):
                        act(st["qsmE"][:, oc, :], st["qp"][oc][:], AFT.Exp,
                            scale=1.0 / SW)
                    sqt = ps_q.tile([128, 512], F32, tag="qps", name="sqt")
                    st["sqt"] = sqt
                    mm(sqt[0:16, :], hm8[:, 0:2, :], st["qsmE"][:, 0:2, :],
                       start=True, stop=False, perf_mode=DR, skip_group_check=True)
                    mm(sqt[0:16, :], hm8[:, 2:4, :], st["qsmE"][:, 2:4, :],
                       start=False, stop=True, perf_mode=DR, skip_group_check=True)
                    rqf = lp1.tile([8, 512], F32R, tag="rqf", name="rqf")
                    st["rqf"] = rqf
                    with nc.allow_low_precision(reason="f32r row for broadcast mm"):
                        nc.vector.reciprocal(rqf[:], sqt[0:8, :])

            q_piece(0, 0)
            Wkv8T = const_tile((128, CC, 1024), F8, Wkv8T_d, "Wkv8T")
            q_piece(0, 1)
            late = load_late_consts()
            WrTb, W1gTb, W2gTb = late["WrTb"], late["W1gTb"], late["W2gTb"]
            u2ct, inv512, ivg8, g2c = (late["u2ct"], late["inv512"],
                                       late["ivg8"], late["g2c"])
            ones8p = late["ones8p"]
            br_c = late.get("br_c")
            w1bb_r = late.get("w1bb_r")
            g2b2_r = late.get("g2b2_r")
            be2_c = late.get("be2_c")
            q_piece(0, 2)

            for ot in range(NT):
                sl = slice(ot * 512, (ot + 1) * 512)
                st = qstate[ot]
                z2c, qsmE, rqf = st["z2c"], st["qsmE"], st["rqf"]
                for s in range(4):
                    ls = slice(s * 128, (s + 1) * 128)
                    slot = s % 2
                    pair = (ot * 2 + s // 2) % 3
                    Ek2, vT2 = Ek2s[pair], vT2s[pair]
                    pr = s
                    # qsm8 = qsmE * (64/Sq) broadcast per head
                    rqbt = ps_m.tile([128, 512], F32, tag="mps", name="rqbt")
                    mm(rqbt[:], hm64[:, pr, :], rqf[:],
                       start=True, stop=True)
                    tt(qsm8[:, pr, sl], qsmE[:, pr, :], rqbt[:],
                       AluOpType.mult)
                    kps = ps_k.tile([128, 512], F32, tag="kps", name="kps")
                    for p in (0, 2):
                        mm(kps[:], z2c[:, p:p + 2, ls], Wkv8T[:, p:p + 2, 0:512],
                           start=(p == 0), stop=(p == 2), perf_mode=DR)
                    vps = ps_m.tile([128, 512], F32, tag="mps", name="vps")
                    for p in (0, 2):
                        mm(vps[:], z2c[:, p:p + 2, ls], Wkv8T[:, p:p + 2, 512:1024],
                           start=(p == 0), stop=(p == 2), perf_mode=DR)
                    act(Ek2[:, slot, :], kps[:], AFT.Exp, scale=1.0 / SW)
                    vdst = vT2[:, slot, :].rearrange("p (pr x) -> p pr x", x=144)[:, :, 0:128]
                    vsrc = vps[:].rearrange("p (pr x) -> p pr x", x=128)
                    if s == 3:
                        act(vdst, vsrc, AFT.Copy)
                    else:
                        nc.vector.tensor_copy(vdst, vsrc)
                    if slot == 1:
                        first = (ot == 0 and s == 1)
                        last = (ot == NT - 1 and s == 3)
                        for pr2 in range(CC):
                            mm(ctxa[:, pr2, :], Ek2[:, :, pr2 * 128:(pr2 + 1) * 128],
                               vT2[:, :, pr2 * 144:pr2 * 144 + 129],
                               start=first, stop=last, perf_mode=DR,
                               skip_group_check=True)
                    if ot + 1 < NT and s < 3:
                        q_piece(ot + 1, s)
                if ot in qstate:
                    del qstate[ot]

            # finalize: normalize ctx rows, build W_combT = ctx_bd^T @ Wr^T in fp8
            p1i.close()
            with ExitStack() as fz:
                ft = fz.enter_context(tc.tile_pool(name="ft", bufs=1))
                ps_t = fz.enter_context(tc.tile_pool(name="ps_t", bufs=2, space="PSUM"))
                rs_l, cbd_l, tps_l, cT_l = [], [], [], []
                for pr in range(CC):
                    rs = ft.tile([128, 1], F32, tag=f"rs{pr}", name=f"rs{pr}")
                    nc.vector.reciprocal(rs[:], ctxa[:, pr, 128:129])
                    rs_l.append(rs)
                    cbd = ft.tile([128, 128], BF16, tag=f"cbd{pr}", name=f"cbd{pr}")
                    nc.vector.memset(cbd[:], 0.0)
                    cbd_l.append(cbd)
                for pr in range(CC):
                    ts(cbd_l[pr][0:64, 0:64], ctxa[0:64, pr, 0:64], rs_l[pr][0:64, :],
                       1.0 / SW, AluOpType.mult, AluOpType.mult)
                    ts(cbd_l[pr][64:128, 64:128], ctxa[64:128, pr, 64:128],
                       rs_l[pr][64:128, :], 1.0 / SW, AluOpType.mult, AluOpType.mult)
                for pr in range(CC):
                    tps = ps_t.tile([128, 128], BF16, tag="tps")
                    nc.tensor.transpose(tps[:], cbd_l[pr][:], identb[:])
                    tps_l.append(tps)
                    cT = ft.tile([128, 128], BF16, tag=f"cT{pr}", name=f"cT{pr}")
                    nc.vector.tensor_copy(cT[:], tps[:])
                    cT_l.append(cT)
                for pr in range(CC):
                    wcps = ps_t.tile([128, 512], F32, tag="wcps")
                    mm(wcps[:], cT_l[pr][:], WrTb[:, pr, :], start=True, stop=True)
                    act(WcT8[:, pr, :], wcps[:], AFT.Copy, scale=SC)

        # ------------- Phase 2: z = Wc qsm + z1, LN1, FFN, LN2 -------------
        # Software-pipelined: stage A (z + LN1 stats + xn) runs one tile ahead
        # of stage B (FFN + LN2 + output) so B's long FFN matmul stretch hides
        # A's LN1 latency chain and A(t+1)'s z matmuls hide B(t)'s LN2 tail.
        with ExitStack() as p2:
            lp2 = p2.enter_context(tc.tile_pool(name="lp2", bufs=2))
            lpx = p2.enter_context(tc.tile_pool(name="lpx", bufs=3))
            lpr = p2.enter_context(tc.tile_pool(name="lpr", bufs=2))
            lpe = p2.enter_context(tc.tile_pool(name="lpe", bufs=3))
            ps_z = p2.enter_context(tc.tile_pool(name="ps_z", bufs=2, space="PSUM"))
            ps_f = p2.enter_context(tc.tile_pool(name="ps_f", bufs=2, space="PSUM"))
            ps_f2 = p2.enter_context(tc.tile_pool(name="ps_f2", bufs=2, space="PSUM"))
            ps_row = p2.enter_context(tc.tile_pool(name="ps_row", bufs=2, space="PSUM"))

            def stage_a(lt):
                sl = slice(lt * 512, (lt + 1) * 512)
                z1r = lp2.tile([128, CC, 512], BF16, tag="z1r", name="z1r")
                nc.sync.dma_start(z1r[:], z1res_d[:, :, sl])
                rows = ps_row.tile([128, 512], F32, tag="rows", name="rows")

                zb = lp2.tile([128, CC, 512], BF16, tag="zb", name="zb")
                zsq = lp2.tile([128, CC, 512], F8, tag="zsq", name="zsq")
                for oc in range(CC):
                    os_ = slice(oc * 128, (oc + 1) * 128)
                    zps = ps_z.tile([128, 512], F32, tag="zps", name="zps")
                    mm(zps[:], WcT8[:, 0:2, os_], qsm8[:, 0:2, sl],
                       start=True, stop=False, perf_mode=DR)
                    mm(zps[:], WcT8[:, 2:4, os_], qsm8[:, 2:4, sl],
                       start=False, stop=True, perf_mode=DR)
                    stt(zb[:, oc, :], zps[:], 1.0 / (SC * SQ), z1r[:, oc, :],
                        AluOpType.mult, AluOpType.add)
                    if br_nz:
                        gp.tensor_scalar(zb[:, oc, :], zb[:, oc, :],
                                         br_c[:, oc:oc + 1], None, AluOpType.add)
                    act(zsq[:, oc, :], zb[:, oc, :], AFT.Square)
                # mean row at partition 32 (bf16 matmul may target 32); the
                # fp8 DoubleRow square-sum must target partition 0
                for oc in range(CC):
                    mm(rows[32:33, :], inv512[:], zb[:, oc, :],
                       start=(oc == 0), stop=(oc == CC - 1), skip_group_check=True)
                mm(rows[0:1, :], ones8p[:, 0:2, 0:1], zsq[:, 0:2, :],
                   start=True, stop=False, perf_mode=DR, skip_group_check=True)
                mm(rows[0:1, :], ones8p[:, 2:4, 0:1], zsq[:, 2:4, :],
                   start=False, stop=True, perf_mode=DR, skip_group_check=True)

                # LN1 rows: 1/sigma = exp(-0.5 ln(var+eps)) keeps every ACT op
                # in the same activation-table set (no table reloads)
                musq = lpr.tile([1, 512], BF16, tag="musq", name="musq")
                act(musq[:], rows[32:33, :], AFT.Square)
                varb = lpr.tile([1, 512], BF16, tag="varb", name="varb")
                stt(varb[:], rows[0:1, :], 1.0 / 512.0, musq[:],
                    AluOpType.mult, AluOpType.subtract)
                lnv = lpr.tile([1, 512], F32, tag="lnv", name="lnv")
                act(lnv[:], varb[:], AFT.Ln, bias=epsA[0:1, :])
                invbr = lpr.tile([1, 512], BF16, tag="invbr", name="invbr")
                act(invbr[:], lnv[:], AFT.Exp, scale=-0.5)
                numur = lpr.tile([1, 512], BF16, tag="numur", name="numur")
                stt(numur[:], rows[32:33, :], -1.0, invbr[:], AluOpType.mult,
                    AluOpType.mult)
                invsb = lp2.tile([128, 512], BF16, tag="invsb", name="invsb")
                gp.partition_broadcast(invsb[:], invbr[:])
                numub = lp2.tile([128, 512], BF16, tag="numub", name="numub")
                gp.partition_broadcast(numub[:], numur[:])

                xn = lpx.tile([128, CC, 512], BF16, tag="xn", name="xn")
                for oc in range(CC):
                    tt(xn[:, oc, :], zb[:, oc, :], invsb[:], AluOpType.mult)
                    tt(xn[:, oc, :], xn[:, oc, :], numub[:], AluOpType.add)
                return sl, rows, xn

            def b2_chunk(st, oc, s1, sq2, drain=False):
                sl2, heh2, negm2b2 = st
                os_ = slice(oc * 128, (oc + 1) * 128)
                f2ps = ps_f2.tile([128, 512], F32, tag="f2ps", name="f2ps")
                for j in range(8):
                    mm(f2ps[:], W2gTb[:, j, os_], heh2[j // 4][:, j % 4, :],
                       start=(j == 0), stop=(j == 7 and not b2_nz))
                if b2_nz:
                    mm(f2ps[:], g2b2_r[:, os_], ones_row[:], start=False, stop=True)
                stt(s1[:, oc, :], negm2b2[:], g2c[:, oc:oc + 1], f2ps[:],
                    AluOpType.mult, AluOpType.add)
                # during the pipeline drain DVE is idle and Pool's slow TT
                # would sit on the critical path
                sqeng = nc.vector if drain else gp
                sqeng.tensor_tensor(sq2[:, oc, :], s1[:, oc, :], s1[:, oc, :],
                                    AluOpType.mult)

            def b2_tail(st, s1, sq2):
                sl2, heh2, negm2b2 = st
                # variance row via fp8 DoubleRow (ivg8 = 1/g2^2 in col 0);
                # the 1/512 is folded into the Ln scale
                e2t = ps_f.tile([128, 512], F32, tag="fps", name="e2t")
                mm(e2t[0:1, :], ivg8[:, 0:2, 0:1], sq2[:, 0:2, :],
                   start=True, stop=False, perf_mode=DR, skip_group_check=True)
                mm(e2t[0:1, :], ivg8[:, 2:4, 0:1], sq2[:, 2:4, :],
                   start=False, stop=True, perf_mode=DR, skip_group_check=True)

                ln2v = lpr.tile([1, 512], F32, tag="ln2v", name="ln2v")
                act(ln2v[:], e2t[0:1, :], AFT.Ln, scale=1.0 / 512.0,
                    bias=epsA[0:1, :])
                inv2br = lpr.tile([1, 512], BF16, tag="inv2br", name="inv2br")
                act(inv2br[:], ln2v[:], AFT.Exp, scale=-0.5)
                invs2b = lp2.tile([128, 512], BF16, tag="invs2b", name="invs2b")
                gp.partition_broadcast(invs2b[:], inv2br[:])

                for oc in range(CC):
                    yo = lp2.tile([128, 512], BF16, tag=f"yo{oc}", name=f"yo{oc}")
                    tt(yo[:], s1[:, oc, :], invs2b[:], AluOpType.mult)
                    if be2_nz:
                        ts(yo[:], yo[:], be2_c[:, oc:oc + 1], None, AluOpType.add)
                    nc.sync.dma_start(outd[oc * 128:(oc + 1) * 128, sl2], yo[:])

            def stage_b(a_st, b_st):
                """FFN1+ELU for tile a_st, with the previous tile's FFN2
                oc-chunks interleaved between FFN1 j-pairs so PE always has
                independent matmuls while the ELU chain drains."""
                if b_st is not None:
                    s1 = lp2.tile([128, CC, 512], BF16, tag="s1", name="s1")
                    sq2 = lp2.tile([128, CC, 512], F8, tag="sq2", name="sq2")
                if a_st is None:
                    for oc in range(CC):
                        b2_chunk(b_st, oc, s1, sq2, drain=True)
                    b2_tail(b_st, s1, sq2)
                    return None
                sl, rows, xn = a_st
                heh = [lp2.tile([128, 4, 512], BF16, tag=f"he{h}", name=f"he{h}")
                       for h in range(2)]
                for j in range(8):
                    fps = ps_f.tile([128, 512], F32, tag="fps", name="fps")
                    js = slice(j * 128, (j + 1) * 128)
                    for cc in range(CC):
                        mm(fps[:], W1gTb[:, cc, js], xn[:, cc, :],
                           start=(cc == 0), stop=(cc == CC - 1 and not b1_nz))
                    if b1_nz:
                        mm(fps[:], w1bb_r[:, js], ones_row[:], start=False, stop=True)
                    Eb = lpe.tile([128, 512], BF16, tag="Eb", name="Eb")
                    act(Eb[:], fps[:], AFT.Exp)
                    ts(Eb[:], Eb[:], 1.0, -1.0, AluOpType.min, AluOpType.add)
                    if j % 2 == 0:
                        # elu in one DVE pass: max(h,0) + (min(exp(h),1)-1)
                        stt(heh[j // 4][:, j % 4, :], fps[:], 0.0, Eb[:],
                            AluOpType.max, AluOpType.add)
                    else:
                        hp = lpe.tile([128, 512], BF16, tag="hp", name="hp")
                        act(hp[:], fps[:], AFT.Relu)
                        tt(heh[j // 4][:, j % 4, :], hp[:], Eb[:], AluOpType.add)
                    if b_st is not None and j % 2 == 1:
                        b2_chunk(b_st, j // 2, s1, sq2)
                for j in range(8):
                    mm(rows[64:65, :], u2ct[:, j:j + 1], heh[j // 4][:, j % 4, :],
                       start=(j == 0), stop=(j == 7), skip_group_check=True)
                negm2 = lpr.tile([1, 512], BF16, tag="negm2", name="negm2")
                ts(negm2[:], rows[64:65, :], -1.0, -B2MEAN_PLACEHOLDER,
                   AluOpType.mult, AluOpType.add)
                negm2b = lp2.tile([128, 512], BF16, tag="negm2b", name="negm2b")
                gp.partition_broadcast(negm2b[:], negm2[:])
                if b_st is not None:
                    b2_tail(b_st, s1, sq2)
                return sl, heh, negm2b

            pa, pb = None, None
            for lt in range(NT):
                cur = stage_a(lt)
                if pa is not None:
                    pb = stage_b(pa, pb)
                pa = cur
            pb = stage_b(pa, pb)
            stage_b(None, pb)

    nc.compile()
    return nc


def _prep_consts(Wq, bq, Wk, bk, Wv, bv, Wr, br, g1, be1, W1, b1, W2, b2, g2, be2):
    import ml_dtypes
    f = np.float32
    F8NP = ml_dtypes.float8_e4m3
    BFNP = ml_dtypes.bfloat16

    def chunkT(a, n):          # [n*128, m] -> [128, n, m]
        return np.ascontiguousarray(a.reshape(n, 128, -1).transpose(1, 0, 2))

    def colsT(v, n):           # [n*128] -> [128, n]
        return np.ascontiguousarray(v.reshape(n, 128).T)

    WqT = np.ascontiguousarray(Wq.T, dtype=f)
    WkvT = np.concatenate([Wk.T, Wv.T], axis=1).astype(f)
    WrT = np.ascontiguousarray(Wr.T, dtype=f)
    W1g = (W1 * g1[None, :]).astype(f)
    W1gT = np.ascontiguousarray(W1g.T)
    W2g = (W2 * g2[:, None]).astype(f)
    W2gT = np.ascontiguousarray(W2g.T)
    w1bb = (W1 @ be1 + b1).astype(f)
    u2 = (W2.sum(axis=0) / 512.0).astype(f)
    ivg = (1.0 / (g2 * g2)).astype(f)          # 1/512 folded into Ln scale
    b2mean = float(np.mean(b2))
    br_eff = (br + Wr @ bv).astype(f)
    ivg8 = np.zeros((128, CC, 16), dtype=f)
    ivg8[:, :, 0] = colsT(ivg, CC)
    ones8p = np.zeros((128, CC, 16), dtype=f)
    ones8p[:, :, 0] = 1.0

    # head mask: channel (cc, p) -> global k-channel cc*128+p -> head //64
    chan = (np.arange(CC)[None, :] * 128 + np.arange(128)[:, None])  # [128, CC]
    head = chan // DK                                                # [128, CC]
    hm8 = np.zeros((128, CC, 16), dtype=f)   # padded to 16 cols for DoubleRow
    for hh in range(8):
        hm8[:, :, hh] = (head == hh)
    hm64 = np.zeros((8, CC, 128), dtype=f)
    for pr in range(CC):
        for hh in range(8):
            hm64[hh, pr, :] = 64.0 * (head[:, pr] == hh)

    consts = {
        "Wq8T": chunkT(WqT * SW, CC).astype(F8NP),
        "Wkv8T": chunkT(WkvT * SW, CC).astype(F8NP),
        "hm8": hm8.astype(F8NP),
        "hm64": hm64,
        "WrTb": chunkT(WrT, CC).astype(BFNP),
        "W1gTb": chunkT(W1gT, CC).astype(BFNP),
        "W2gTb": chunkT(W2gT, 8).astype(BFNP),
        "u2ct": colsT(u2, 8).astype(BFNP),
        "inv512": np.full((128, 1), 1.0 / 512.0, dtype=f).astype(BFNP),
        "ivg8": ivg8.astype(F8NP),
        "ones8p": ones8p.astype(F8NP),
        "g2c": colsT(g2.astype(f), CC),
        "identb": np.eye(128, dtype=f).astype(BFNP),
        "epsA": np.full((1, 1), EPS, dtype=f),
        "ones_row": np.ones((1, 512), dtype=f),
        "bq_r": bq.reshape(1, 512).astype(f),
        "br_c": colsT(br_eff, CC),
        "w1bb_r": w1bb.reshape(1, 1024).astype(f),
        "g2b2_r": (g2 * b2).reshape(1, 512).astype(f),
        "be2_c": colsT(be2.astype(f), CC),
    }
    flags = (bool(np.any(bq)), bool(np.any(br_eff)), bool(np.any(w1bb)),
             bool(np.any(b2)), bool(np.any(be2)))
    return consts, b2mean, flags


def kernel(**inputs):
    global LAST_RESULT, B2MEAN_PLACEHOLDER
    import ml_dtypes
    F8NP = ml_dtypes.float8_e4m3
    BFNP = ml_dtypes.bfloat16
    z1 = np.asarray(inputs["z1"], dtype=np.float32)
    z2 = np.asarray(inputs["z2"], dtype=np.float32)
    consts, b2mean, flags = _prep_consts(
        *[np.asarray(inputs[k], dtype=np.float32) for k in
          ["Wq", "bq", "Wk", "bk", "Wv", "bv", "Wr", "br", "g1", "be1",
           "W1", "b1", "W2", "b2", "g2", "be2"]])

    key = ("prog", flags, round(b2mean * 1e9))
    if key not in _CACHE:
        B2MEAN_PLACEHOLDER = b2mean
        _CACHE.clear()
        _CACHE[key] = _build_program(flags)
    nc = _CACHE[key]

    def rearr(a):             # [C, L] -> [128, CC, L]
        return np.ascontiguousarray(a.reshape(CC, 128, L).transpose(1, 0, 2))

    in_maps = []
    for b in range(B):
        m = dict(consts)
        m["z1_8"] = rearr(z1[b]).astype(F8NP)
        m["z2_8"] = rearr(z2[b]).astype(F8NP)
        m["z1res"] = rearr(z1[b]).astype(BFNP)
        in_maps.append(m)

    import os
    trace = bool(int(os.environ.get("KERNEL_TRACE", "0")))
    res = run_bass_kernel_spmd(nc, in_maps, list(range(B)), trace=trace)
    LAST_RESULT = res
    out = np.stack([np.asarray(res.results[b]["out"]).astype(np.float32)
                    for b in range(B)], axis=0)
    return out


B2MEAN_PLACEHOLDER = 0.0
